# revision 1
# baseline (speedup 1.0000x reference)
"""JunctionGNN Trainium2 kernel: 3-layer GCN + edge MLP over 100k nodes / 1.6M edges.

Sharding: edges sorted by destination (col); core k owns a contiguous slice of
12544 dst nodes and its incoming edges. Per 128-node dst block, edges are
grouped by source-row table chunk (4 chunks, int16-indexable) and padded to
uniform per-chunk tile counts so one NEFF serves all 8 cores.

Per GCN layer: each core computes g = (h @ W) * dinv for its slice, AllGathers
the full g table to DRAM, then per dst block: batched dma_gather of g[row]
(one call per table chunk), a single broadcast is_equal builds all one-hot
[128e x 128n] tiles, and the PE scatter-accumulates agg = onehot.T @ g_rows
into PSUM. Epilogue: self-loop, dinv, bias, LayerNorm, ReLU, residual.

Edge MLP runs feature-major: e.T from host-transposed edge_attr; u_a/u_b =
h @ W_ep1 halves are packed into one bf16 [N,128] table, AllGathered, gathered
per edge by row/col, and folded into the er PSUM with matmuls against identity.
"""

import os
import numpy as np
from ml_dtypes import bfloat16

import concourse.bass as bass
import concourse.bacc as bacc
import concourse.mybir as mybir
import concourse.tile as tile
from concourse import bass_utils
from concourse import library_config

P = 128
H = 64
EPS = 1e-5
NCORES = 8
NCHUNK = 4


def _ceil(a, b):
    return (a + b - 1) // b


def _prep(inputs):
    """Host-side index preprocessing + per-core input construction."""
    x = np.asarray(inputs["x"], np.float32)
    edge_attr = np.asarray(inputs["edge_attr"], np.float32)
    ei = np.asarray(inputs["edge_index"]).astype(np.int64)
    N = x.shape[0]
    E = ei.shape[1]
    EA = edge_attr.shape[1]

    row, col = ei[0], ei[1]
    perm = np.argsort(col, kind="stable")
    row_s = row[perm]
    col_s = col[perm]

    nslice = _ceil(_ceil(N, NCORES), P) * P          # nodes per core slice
    nblk = nslice // P                               # dst blocks per core
    NG = nslice * NCORES                             # gathered table rows
    CH = NG // NCHUNK                                # table chunk rows (int16-safe)
    assert CH % P == 0 and CH <= 32767

    deg = np.zeros(NG, np.float32)
    deg[:N] = np.bincount(col, minlength=N).astype(np.float32)
    deg += 1.0

    nblk_g = nblk * NCORES
    block_lo = np.searchsorted(col_s, np.arange(0, NG, P))
    block_hi = np.searchsorted(col_s, np.arange(P, NG + P, P))
    rchunk_s = row_s // CH

    # per (block, chunk) counts -> uniform per-chunk tile counts Tc
    cnt = np.zeros((nblk_g, NCHUNK), np.int64)
    for gb in range(nblk_g):
        lo, hi = int(block_lo[gb]), int(block_hi[gb])
        if hi > lo:
            cnt[gb] = np.bincount(rchunk_s[lo:hi], minlength=NCHUNK)
    Tc = [int(_ceil(int(cnt[:, c].max()), P)) for c in range(NCHUNK)]
    Tc = [max(t, 1) for t in Tc]
    TT = sum(Tc)
    C0 = np.cumsum([0] + Tc)                          # tile offsets per chunk
    Ec = nblk * TT * P

    in_maps, meta = [], []
    for k in range(NCORES):
        idx16 = np.zeros((nblk, 16, TT * 8), np.int16)
        cidx16 = np.zeros((nblk, 16, TT * 8), np.int16)  # replicated to 128 rows below
        colrel = np.full((nblk, P, TT), -1.0, np.float32)
        eaT = np.zeros((Ec, EA), np.float32)
        flat_pos, sort_idx = [], []
        for b in range(nblk):
            gb = k * nblk + b
            lo, hi = int(block_lo[gb]), int(block_hi[gb])
            n = hi - lo
            if n == 0:
                continue
            rc = rchunk_s[lo:hi]
            order = np.argsort(rc, kind="stable")
            rows = row_s[lo:hi][order]
            cols = col_s[lo:hi][order]
            ccnt = np.bincount(rc, minlength=NCHUNK)
            # flat position within the padded block for each edge
            j_in_chunk = np.arange(n) - np.repeat(np.cumsum(np.concatenate([[0], ccnt[:-1]])), ccnt)
            jpos = C0[rc[order]] * P + j_in_chunk
            # row indices: within-call index i = jpos - C0[c]*128
            i_call = jpos - C0[rc[order]] * P
            idx16[b, i_call % 16, C0[rc[order]] * 8 + i_call // 16] = (rows - rc[order] * CH).astype(np.int16)
            # col indices (slice-local, gathered from this core's own u table)
            cidx16[b, jpos % 16, jpos // 16] = (cols - k * nslice).astype(np.int16)
            colrel[b, jpos % P, jpos // P] = (cols - gb * P).astype(np.float32)
            base = b * TT * P
            eaT[base + jpos] = edge_attr[perm[lo:hi][order]]
            flat_pos.append(base + jpos)
            sort_idx.append(np.arange(lo, hi)[order])
        meta.append(
            (
                np.concatenate(flat_pos) if flat_pos else np.zeros(0, np.int64),
                np.concatenate(sort_idx) if sort_idx else np.zeros(0, np.int64),
            )
        )

        xT = np.zeros((2, nslice), np.float32)
        s0, s1 = k * nslice, min((k + 1) * nslice, N)
        if s1 > s0:
            xT[:, : s1 - s0] = x[s0:s1].T
        degs = deg[k * nslice : (k + 1) * nslice].reshape(nblk, P).T.copy()

        in_maps.append(
            {
                "idx16": np.tile(idx16, (1, 8, 1)).reshape(nblk * 128, TT * 8),
                "cidx16": np.tile(cidx16, (1, 8, 1)).reshape(nblk * 128, TT * 8),
                "colrel": colrel.reshape(nblk * P, TT),
                "eaT": np.ascontiguousarray(eaT.T),
                "xT": xT,
                "deg": degs,
            }
        )

    W_ep1 = np.asarray(inputs["W_ep1"], np.float32)
    shared = {
        "W_node": np.asarray(inputs["W_node"], np.float32),
        "b_node_bc": np.broadcast_to(np.asarray(inputs["b_node"], np.float32), (P, H)).copy(),
        "W_edge": np.asarray(inputs["W_edge"], np.float32),
        "b_edge_c": np.asarray(inputs["b_edge"], np.float32).reshape(H, 1),
        "W1a": np.ascontiguousarray(W_ep1[0:H]),
        "W1b": np.ascontiguousarray(W_ep1[H : 2 * H]),
        "W1c": np.ascontiguousarray(W_ep1[2 * H : 3 * H]),
        "b_ep1_c": np.asarray(inputs["b_ep1"], np.float32).reshape(H, 1),
        "W_ep2": np.asarray(inputs["W_ep2"], np.float32),
        "b_ep2_c": np.asarray(inputs["b_ep2"], np.float32).reshape(H // 2, 1),
        "W_heads": np.ascontiguousarray(
            np.concatenate(
                [
                    np.asarray(inputs["W_from"], np.float32),
                    np.asarray(inputs["W_to"], np.float32),
                    np.asarray(inputs["W_turn"], np.float32),
                ],
                axis=1,
            )
        ),
        "b_heads_c": np.array(
            [inputs["b_from"][0], inputs["b_to"][0], inputs["b_turn"][0]], np.float32
        ).reshape(3, 1),
        "id_f32": np.eye(P, dtype=np.float32),
        "id_bf16": np.eye(P, dtype=bfloat16),
        "iota": np.tile(np.arange(P, dtype=np.float32), (P, 1)),
    }
    for i in range(3):
        shared[f"Wc{i}"] = np.ascontiguousarray(np.asarray(inputs["W_conv"], np.float32)[i])
        shared[f"bc{i}"] = np.broadcast_to(
            np.asarray(inputs["b_conv"], np.float32)[i], (P, H)
        ).copy()
        shared[f"lg{i}"] = np.broadcast_to(
            np.asarray(inputs["ln_g"], np.float32)[i], (P, H)
        ).copy()
        shared[f"lb{i}"] = np.broadcast_to(
            np.asarray(inputs["ln_b"], np.float32)[i], (P, H)
        ).copy()
    for m in in_maps:
        m.update(shared)

    dims = dict(N=N, E=E, EA=EA, nslice=nslice, nblk=nblk, NG=NG, CH=CH,
                Tc=Tc, TT=TT, Ec=Ec)
    return dims, in_maps, meta, perm


def _build(dims):
    f32 = mybir.dt.float32
    bf16 = mybir.dt.bfloat16
    i16 = mybir.dt.int16
    AX = mybir.AxisListType
    OP = mybir.AluOpType
    AF = mybir.ActivationFunctionType

    nslice, nblk, NG, CH, Tc, TT, Ec, EA = (
        dims["nslice"], dims["nblk"], dims["NG"], dims["CH"],
        dims["Tc"], dims["TT"], dims["Ec"], dims["EA"],
    )
    C0 = np.cumsum([0] + list(Tc))
    rg = [list(range(NCORES))]

    nc = bacc.Bacc(
        "TRN2", target_bir_lowering=False, debug=False,
        enable_asserts=False, num_devices=NCORES,
    )

    d_idx16 = nc.dram_tensor("idx16", [nblk * 128, TT * 8], i16, kind="ExternalInput").ap()
    d_cidx16 = nc.dram_tensor("cidx16", [nblk * 128, TT * 8], i16, kind="ExternalInput").ap()
    d_colrel = nc.dram_tensor("colrel", [nblk * P, TT], f32, kind="ExternalInput").ap()
    d_eaT = nc.dram_tensor("eaT", [EA, Ec], f32, kind="ExternalInput").ap()
    d_xT = nc.dram_tensor("xT", [2, nslice], f32, kind="ExternalInput").ap()
    d_deg = nc.dram_tensor("deg", [P, nblk], f32, kind="ExternalInput").ap()

    din = {}
    consts = [
        ("W_node", [2, H], f32), ("b_node_bc", [P, H], f32),
        ("W_edge", [EA, H], f32), ("b_edge_c", [H, 1], f32),
        ("W1a", [H, H], f32), ("W1b", [H, H], f32), ("W1c", [H, H], f32),
        ("b_ep1_c", [H, 1], f32), ("W_ep2", [H, H // 2], f32),
        ("b_ep2_c", [H // 2, 1], f32), ("W_heads", [H // 2, 3], f32),
        ("b_heads_c", [3, 1], f32), ("id_f32", [P, P], f32),
        ("id_bf16", [P, P], bf16), ("iota", [P, P], f32),
    ]
    for i in range(3):
        consts += [(f"Wc{i}", [H, H], f32), (f"bc{i}", [P, H], f32),
                   (f"lg{i}", [P, H], f32), (f"lb{i}", [P, H], f32)]
    for name, shp, dt in consts:
        din[name] = nc.dram_tensor(name, shp, dt, kind="ExternalInput").ap()

    d_out = nc.dram_tensor("out", [3, Ec], f32, kind="ExternalOutput").ap()

    ag_g_in = nc.dram_tensor("ag_g_in", [nslice, H], f32, kind="Internal").ap()
    g_full = nc.dram_tensor("g_full", [NG, H], f32, kind="Internal", addr_space="Shared").ap()
    ag_u_in = nc.dram_tensor("ag_u_in", [nslice, 2 * H], bf16, kind="Internal").ap()
    u_full = nc.dram_tensor("u_full", [NG, 2 * H], bf16, kind="Internal", addr_space="Shared").ap()

    with tile.TileContext(nc) as tc:
        nc.gpsimd.load_library(library_config.mlp)
        cp = tc.alloc_tile_pool(name="consts", bufs=1)
        sb = {}
        for name, shp, dt in consts:
            t = cp.tile(shp, dt, name=f"sb_{name}")
            nc.sync.dma_start(out=t[:], in_=din[name])
            sb[name] = t

        xT_sb = cp.tile([2, nslice], f32, name="xT_sb")
        nc.sync.dma_start(out=xT_sb[:], in_=d_xT)
        deg_sb = cp.tile([P, nblk], f32, name="deg_sb")
        nc.sync.dma_start(out=deg_sb[:], in_=d_deg)
        dinv_sb = cp.tile([P, nblk], f32, name="dinv_sb")
        nc.vector.reciprocal(out=dinv_sb[:], in_=deg_sb[:])
        nc.scalar.sqrt(out=dinv_sb[:], in_=dinv_sb[:])

        hbuf = [cp.tile([P, nblk * H], f32, name=f"hbuf{j}") for j in range(2)]
        gbuf = cp.tile([P, nblk * H], f32, name="gbuf")

        # ---- h0 = relu(x @ W_node + b_node)
        with tc.tile_pool(name="p0", bufs=4, space="PSUM") as pp0:
            for b in range(nblk):
                ps = pp0.tile([P, H], f32, name="h0ps")
                nc.tensor.matmul(
                    out=ps[:], lhsT=xT_sb[:, b * P : (b + 1) * P],
                    rhs=sb["W_node"][:], start=True, stop=True,
                )
                hb = hbuf[0][:, b * H : (b + 1) * H]
                nc.vector.tensor_tensor(out=hb, in0=ps[:], in1=sb["b_node_bc"][:], op=OP.add)
                nc.vector.tensor_scalar_max(hb, hb, 0.0)

        # ---- 3 GCN layers
        for li in range(3):
            hc = hbuf[li % 2]
            hn = hbuf[(li + 1) % 2]
            with (
                tc.tile_pool(name=f"gprep{li}", bufs=3, space="PSUM") as gp,
                tc.tile_pool(name=f"gprep_sb{li}", bufs=3) as gs,
            ):
                for b in range(nblk):
                    hT_ps = gp.tile([H, P], f32, name="hT_ps")
                    nc.tensor.matmul(
                        out=hT_ps[:], lhsT=hc[:, b * H : (b + 1) * H],
                        rhs=sb["id_f32"][:], start=True, stop=True,
                    )
                    hT = gs.tile([H, P], f32, name="hT")
                    nc.any.tensor_copy(out=hT[:], in_=hT_ps[:])
                    g_ps = gp.tile([P, H], f32, name="g_ps")
                    nc.tensor.matmul(
                        out=g_ps[:], lhsT=hT[:], rhs=sb[f"Wc{li}"][:],
                        start=True, stop=True,
                    )
                    gb = gbuf[:, b * H : (b + 1) * H]
                    nc.vector.tensor_scalar(
                        out=gb, in0=g_ps[:], scalar1=dinv_sb[:, b : b + 1],
                        scalar2=None, op0=OP.mult,
                    )
                    nc.sync.dma_start(out=ag_g_in[b * P : (b + 1) * P, :], in_=gb)

            nc.gpsimd.collective_compute(
                "AllGather", OP.bypass, replica_groups=rg,
                ins=[ag_g_in], outs=[g_full],
            )

            with (
                tc.tile_pool(name=f"sweep{li}", bufs=3) as sp,
                tc.tile_pool(name=f"sweep_ps{li}", bufs=4, space="PSUM") as spp,
                tc.tile_pool(name=f"epi{li}", bufs=2) as ep,
            ):
                for b in range(nblk):
                    idxs = sp.tile([128, TT * 8], i16, name="idxs")
                    nc.sync.dma_start(out=idxs[:], in_=d_idx16[b * 128 : (b + 1) * 128, :])
                    crel = sp.tile([P, TT], f32, name="crel")
                    nc.sync.dma_start(out=crel[:], in_=d_colrel[b * P : (b + 1) * P, :])
                    gt = sp.tile([P, TT, H], f32, name="gt")
                    for c in range(NCHUNK):
                        nc.gpsimd.dma_gather(
                            out_ap=gt[:, C0[c] : C0[c + 1], :],
                            in_ap=g_full[c * CH : (c + 1) * CH, :],
                            idxs_ap=idxs[:, C0[c] * 8 : C0[c + 1] * 8],
                            num_idxs=Tc[c] * P,
                            num_idxs_reg=Tc[c] * P,
                            elem_size=H,
                            single_packet=False,
                        )
                    oh = sp.tile([P, TT, P], f32, name="oh")
                    nc.any.tensor_tensor(
                        out=oh[:],
                        in0=sb["iota"][:].rearrange("p (o f) -> p o f", o=1).to_broadcast([P, TT, P]),
                        in1=crel[:].rearrange("p (t o) -> p t o", o=1).to_broadcast([P, TT, P]),
                        op=OP.is_equal,
                    )
                    agg = spp.tile([P, H], f32, name="agg")
                    for t in range(TT):
                        nc.tensor.matmul(
                            out=agg[:], lhsT=oh[:, t, :], rhs=gt[:, t, :],
                            start=(t == 0), stop=(t == TT - 1),
                        )
                    # epilogue: self-loop, dinv, bias, LN, relu, residual
                    pre = ep.tile([P, H], f32, name="pre")
                    nc.vector.tensor_tensor(
                        out=pre[:], in0=agg[:], in1=gbuf[:, b * H : (b + 1) * H], op=OP.add
                    )
                    nc.vector.tensor_scalar(
                        out=pre[:], in0=pre[:], scalar1=dinv_sb[:, b : b + 1],
                        scalar2=None, op0=OP.mult,
                    )
                    nc.vector.tensor_tensor(out=pre[:], in0=pre[:], in1=sb[f"bc{li}"][:], op=OP.add)
                    mu = ep.tile([P, 1], f32, name="mu")
                    nc.vector.tensor_reduce(out=mu[:], in_=pre[:], axis=AX.X, op=OP.add)
                    nc.vector.tensor_scalar_mul(mu[:], mu[:], 1.0 / H)
                    xc = ep.tile([P, H], f32, name="xc")
                    nc.vector.tensor_scalar(
                        out=xc[:], in0=pre[:], scalar1=mu[:, 0:1], scalar2=None, op0=OP.subtract
                    )
                    sq = ep.tile([P, H], f32, name="sq")
                    vs = ep.tile([P, 1], f32, name="vs")
                    nc.scalar.activation(
                        out=sq[:], in_=xc[:], func=AF.Square, accum_out=vs[:]
                    )
                    nc.vector.tensor_scalar(
                        out=vs[:], in0=vs[:], scalar1=1.0 / H, scalar2=EPS,
                        op0=OP.mult, op1=OP.add,
                    )
                    nc.vector.reciprocal(out=vs[:], in_=vs[:])
                    nc.scalar.sqrt(out=vs[:], in_=vs[:])
                    nc.vector.tensor_scalar(
                        out=xc[:], in0=xc[:], scalar1=vs[:, 0:1], scalar2=None, op0=OP.mult
                    )
                    nc.vector.tensor_tensor(out=xc[:], in0=xc[:], in1=sb[f"lg{li}"][:], op=OP.mult)
                    nc.vector.tensor_tensor(out=xc[:], in0=xc[:], in1=sb[f"lb{li}"][:], op=OP.add)
                    nc.vector.tensor_scalar_max(xc[:], xc[:], 0.0)
                    nc.vector.tensor_tensor(
                        out=hn[:, b * H : (b + 1) * H], in0=xc[:],
                        in1=hc[:, b * H : (b + 1) * H], op=OP.add,
                    )

        # ---- u table [u_a | u_b] (bf16) + AllGather
        hfin = hbuf[1]
        with (
            tc.tile_pool(name="uprep", bufs=3, space="PSUM") as up,
            tc.tile_pool(name="uprep_sb", bufs=3) as us,
        ):
            for b in range(nblk):
                hT_ps = up.tile([H, P], f32, name="uhT_ps")
                nc.tensor.matmul(
                    out=hT_ps[:], lhsT=hfin[:, b * H : (b + 1) * H],
                    rhs=sb["id_f32"][:], start=True, stop=True,
                )
                hT = us.tile([H, P], f32, name="uhT")
                nc.any.tensor_copy(out=hT[:], in_=hT_ps[:])
                u_bf = us.tile([P, 2 * H], bf16, name="u_bf")
                for j, wname in enumerate(("W1a", "W1b")):
                    u_ps = up.tile([P, H], f32, name="u_ps")
                    nc.tensor.matmul(
                        out=u_ps[:], lhsT=hT[:], rhs=sb[wname][:], start=True, stop=True
                    )
                    nc.any.tensor_copy(out=u_bf[:, j * H : (j + 1) * H], in_=u_ps[:])
                nc.sync.dma_start(out=ag_u_in[b * P : (b + 1) * P, :], in_=u_bf[:])
        nc.gpsimd.collective_compute(
            "AllGather", OP.bypass, replica_groups=rg, ins=[ag_u_in], outs=[u_full]
        )

        # ---- edge MLP (feature-major)
        tile_groups = [(c, min(4, TT - c)) for c in range(0, TT, 4)]
        with (
            tc.tile_pool(name="fsweep", bufs=3) as fp,
            tc.tile_pool(name="fsweep_ps", bufs=2, space="PSUM") as fpp,
        ):
            for b in range(nblk):
                idxs = fp.tile([128, TT * 8], i16, name="fidxs")
                nc.sync.dma_start(out=idxs[:], in_=d_idx16[b * 128 : (b + 1) * 128, :])
                cidx = fp.tile([128, TT * 8], i16, name="fcidx")
                nc.sync.dma_start(out=cidx[:], in_=d_cidx16[b * 128 : (b + 1) * 128, :])
                uar = fp.tile([P, TT, 2 * H], bf16, name="uar")
                for c in range(NCHUNK):
                    nc.gpsimd.dma_gather(
                        out_ap=uar[:, C0[c] : C0[c + 1], :],
                        in_ap=u_full[c * CH : (c + 1) * CH, :],
                        idxs_ap=idxs[:, C0[c] * 8 : C0[c + 1] * 8],
                        num_idxs=Tc[c] * P,
                        num_idxs_reg=Tc[c] * P,
                        elem_size=2 * H,
                        single_packet=False,
                    )
                ubr = fp.tile([P, TT, 2 * H], bf16, name="ubr")
                nc.gpsimd.dma_gather(
                    out_ap=ubr[:],
                    in_ap=ag_u_in[:],
                    idxs_ap=cidx[:],
                    num_idxs=TT * P,
                    num_idxs_reg=TT * P,
                    elem_size=2 * H,
                    single_packet=False,
                )
                for c0g, csz in tile_groups:
                    Ech = csz * P
                    col0 = b * TT * P + c0g * P
                    ea = fp.tile([EA, 4 * P], f32, name="ea")
                    nc.sync.dma_start(out=ea[:, :Ech], in_=d_eaT[:, col0 : col0 + Ech])
                    e_ps = fpp.tile([H, 4 * P], f32, name="e_ps")
                    nc.tensor.matmul(
                        out=e_ps[:, :Ech], lhsT=sb["W_edge"][:], rhs=ea[:, :Ech],
                        start=True, stop=True,
                    )
                    eT = fp.tile([H, 4 * P], f32, name="eT")
                    nc.scalar.activation(
                        out=eT[:, :Ech], in_=e_ps[:, :Ech], func=AF.Relu,
                        bias=sb["b_edge_c"][:, 0:1],
                    )
                    er_ps = fpp.tile([H, 4 * P], f32, name="er_ps")
                    nc.tensor.matmul(
                        out=er_ps[:, :Ech], lhsT=sb["W1c"][:], rhs=eT[:, :Ech],
                        start=True, stop=False,
                    )
                    for tt in range(csz):
                        t = c0g + tt
                        nc.tensor.matmul(
                            out=er_ps[:, tt * P : (tt + 1) * P], lhsT=uar[:, t, 0:H],
                            rhs=sb["id_bf16"][:], start=False, stop=False,
                            skip_group_check=True,
                        )
                        nc.tensor.matmul(
                            out=er_ps[:, tt * P : (tt + 1) * P], lhsT=ubr[:, t, H : 2 * H],
                            rhs=sb["id_bf16"][:], start=False, stop=(tt == csz - 1),
                            skip_group_check=True,
                        )
                    erT = fp.tile([H, 4 * P], f32, name="erT")
                    nc.scalar.activation(
                        out=erT[:, :Ech], in_=er_ps[:, :Ech], func=AF.Relu,
                        bias=sb["b_ep1_c"][:, 0:1],
                    )
                    er2_ps = fpp.tile([H // 2, 4 * P], f32, name="er2_ps")
                    nc.tensor.matmul(
                        out=er2_ps[:, :Ech], lhsT=sb["W_ep2"][:], rhs=erT[:, :Ech],
                        start=True, stop=True,
                    )
                    er2 = fp.tile([H // 2, 4 * P], f32, name="er2")
                    nc.scalar.activation(
                        out=er2[:, :Ech], in_=er2_ps[:, :Ech], func=AF.Relu,
                        bias=sb["b_ep2_c"][:, 0:1],
                    )
                    s_ps = fpp.tile([3, 4 * P], f32, name="s_ps")
                    nc.tensor.matmul(
                        out=s_ps[:, :Ech], lhsT=sb["W_heads"][:], rhs=er2[:, :Ech],
                        start=True, stop=True,
                    )
                    s_sb = fp.tile([3, 4 * P], f32, name="s_sb")
                    nc.vector.tensor_scalar(
                        out=s_sb[:, :Ech], in0=s_ps[:, :Ech],
                        scalar1=sb["b_heads_c"][:, 0:1], scalar2=None, op0=OP.add,
                    )
                    nc.sync.dma_start(out=d_out[:, col0 : col0 + Ech], in_=s_sb[:, :Ech])
        cp.release()
    nc.compile()
    return nc


def kernel(**inputs):
    dims, in_maps, meta, perm = _prep(inputs)
    nc = _build(dims)
    res = bass_utils.run_bass_kernel_spmd(nc, in_maps, core_ids=list(range(NCORES)))
    E = dims["E"]
    s_sorted = np.zeros((3, E), np.float32)
    for k in range(NCORES):
        flat_pos, sort_idx = meta[k]
        if len(sort_idx):
            s_sorted[:, sort_idx] = res.results[k]["out"][:, flat_pos]
    s = np.zeros((3, E), np.float32)
    s[:, perm] = s_sorted
    return s[0], s[1], s[2]



# revision 23
# speedup vs baseline: 2.3255x; 2.3255x over previous
"""JunctionGNN Trainium2 kernel: 3-layer GCN + edge MLP over 100k nodes / 1.6M edges.

Sharding: edges sorted by destination (col); core k owns a contiguous slice of
12544 dst nodes and its incoming edges. Per 128-node dst block, edges are
grouped by source-row table chunk (4 chunks, int16-indexable) and padded to
uniform per-chunk tile counts so one NEFF serves all 8 cores. Pad slots carry
index -1, which the gather ucode trims (no descriptors emitted).

Per GCN layer: each core computes g = (h @ W) * dinv for its slice, AllGathers
a bf16 table [NG, 128] (g in the low half), then per dst block: one dma_gather
per table chunk striped across the 4 SWDGE queues (4 Q7 core pairs run
concurrently), a broadcast is_equal builds bf16 one-hot [128e x 128n] tiles,
and the PE scatter-accumulates agg = onehot.T @ g_rows into PSUM (all-bf16
matmuls). Epilogue: self-loop, dinv, bias, LayerNorm, ReLU, residual in f32.

Edge MLP runs feature-major in bf16: u_a[row] is gathered from the AllGathered
[u_a|u_b] table (same queue striping) and folded via identity matmuls;
u_b[col] needs no gather - cols are block-local, so a transposed one-hot
(built by a rank-1 PSUM broadcast of colrelT + is_equal) matmuls against the
block's resident u_b rows.
"""

import os
import numpy as np
from ml_dtypes import bfloat16

import concourse.bass as bass
import concourse.bacc as bacc
import concourse.mybir as mybir
import concourse.tile as tile
from concourse import bass_utils
from concourse import library_config

P = 128
H = 64
EPS = 1e-5
NCORES = 8
NCHUNK = 4
NSWQ = int(os.environ.get("KERNEL_NSWQ", "4"))  # SWDGE queues (1..4)
UB_OHT = os.environ.get("KERNEL_UB", "oht") == "oht"  # u_b via transposed one-hot
TRIM = os.environ.get("KERNEL_TRIM", "1") == "1"      # -1 pad (trimmed) vs 0 pad


def _ceil(a, b):
    return (a + b - 1) // b


def _prep(inputs):
    """Host-side index preprocessing + per-core input construction."""
    x = np.asarray(inputs["x"], np.float32)
    edge_attr = np.asarray(inputs["edge_attr"], np.float32)
    ei = np.asarray(inputs["edge_index"]).astype(np.int64)
    N = x.shape[0]
    E = ei.shape[1]
    EA = edge_attr.shape[1]

    row, col = ei[0], ei[1]
    perm = np.argsort(col, kind="stable")
    row_s = row[perm]
    col_s = col[perm]

    nslice = _ceil(_ceil(N, NCORES), P) * P          # nodes per core slice
    nblk = nslice // P                               # dst blocks per core
    NG = nslice * NCORES                             # gathered table rows
    CH = NG // NCHUNK                                # table chunk rows (int16-safe)
    assert CH % P == 0 and CH <= 32767

    deg = np.zeros(NG, np.float32)
    deg[:N] = np.bincount(col, minlength=N).astype(np.float32)
    deg += 1.0

    nblk_g = nblk * NCORES
    block_lo = np.searchsorted(col_s, np.arange(0, NG, P))
    block_hi = np.searchsorted(col_s, np.arange(P, NG + P, P))
    rchunk_s = row_s // CH

    # per (block, chunk) counts -> uniform per-chunk tile counts Tc
    cnt = np.zeros((nblk_g, NCHUNK), np.int64)
    for gb in range(nblk_g):
        lo, hi = int(block_lo[gb]), int(block_hi[gb])
        if hi > lo:
            cnt[gb] = np.bincount(rchunk_s[lo:hi], minlength=NCHUNK)
    Tc = [int(_ceil(int(cnt[:, c].max()), P)) for c in range(NCHUNK)]
    Tc = [max(t, 1) for t in Tc]
    TT = sum(Tc)
    C0 = np.cumsum([0] + Tc)                          # tile offsets per chunk
    Ec = nblk * TT * P

    in_maps, meta = [], []
    for k in range(NCORES):
        pad = -1 if TRIM else 0
        idx16 = np.full((nblk, 16, TT * 8), pad, np.int16)  # -1 pads are trimmed
        cidx16 = np.full((nblk, 16, TT * 8), pad, np.int16)
        colrel = np.full((nblk, P, TT), -1.0, np.float32)
        colrelT = np.full((nblk, TT * P), -1.0, np.float32)
        eaT = np.zeros((Ec, EA), np.float32)
        flat_pos, sort_idx = [], []
        for b in range(nblk):
            gb = k * nblk + b
            lo, hi = int(block_lo[gb]), int(block_hi[gb])
            n = hi - lo
            if n == 0:
                continue
            rc = rchunk_s[lo:hi]
            order = np.argsort(rc, kind="stable")
            rows = row_s[lo:hi][order]
            cols = col_s[lo:hi][order]
            ccnt = np.bincount(rc, minlength=NCHUNK)
            # flat position within the padded block for each edge
            j_in_chunk = np.arange(n) - np.repeat(np.cumsum(np.concatenate([[0], ccnt[:-1]])), ccnt)
            jpos = C0[rc[order]] * P + j_in_chunk
            # row indices: within-call index i = jpos - C0[c]*128
            i_call = jpos - C0[rc[order]] * P
            idx16[b, i_call % 16, C0[rc[order]] * 8 + i_call // 16] = (rows - rc[order] * CH).astype(np.int16)
            cidx16[b, jpos % 16, jpos // 16] = (cols - k * nslice).astype(np.int16)
            crel = (cols - gb * P).astype(np.float32)
            colrel[b, jpos % P, jpos // P] = crel
            colrelT[b, jpos] = crel
            base = b * TT * P
            eaT[base + jpos] = edge_attr[perm[lo:hi][order]]
            flat_pos.append(base + jpos)
            sort_idx.append(np.arange(lo, hi)[order])
        meta.append(
            (
                np.concatenate(flat_pos) if flat_pos else np.zeros(0, np.int64),
                np.concatenate(sort_idx) if sort_idx else np.zeros(0, np.int64),
            )
        )

        xT = np.zeros((2, nslice), np.float32)
        s0, s1 = k * nslice, min((k + 1) * nslice, N)
        if s1 > s0:
            xT[:, : s1 - s0] = x[s0:s1].T
        degs = deg[k * nslice : (k + 1) * nslice].reshape(nblk, P).T.copy()

        in_maps.append(
            {
                "idx16": np.tile(idx16, (1, 8, 1)).reshape(nblk * 128, TT * 8),
                **({} if UB_OHT else {"cidx16": np.tile(cidx16, (1, 8, 1)).reshape(nblk * 128, TT * 8)}),
                "colrel": colrel.reshape(nblk * P, TT).astype(bfloat16),
                "colrelT": colrelT.astype(bfloat16),
                "eaT": np.ascontiguousarray(eaT.T).astype(bfloat16),
                "xT": xT,
                "deg": degs,
            }
        )

    W_ep1 = np.asarray(inputs["W_ep1"], np.float32)
    shared = {
        "W_node": np.asarray(inputs["W_node"], np.float32),
        "b_node_bc": np.broadcast_to(np.asarray(inputs["b_node"], np.float32), (P, H)).copy(),
        "W_edge": np.asarray(inputs["W_edge"], np.float32).astype(bfloat16),
        "b_edge_c": np.asarray(inputs["b_edge"], np.float32).reshape(H, 1),
        "W1a": np.ascontiguousarray(W_ep1[0:H]).astype(bfloat16),
        "W1b": np.ascontiguousarray(W_ep1[H : 2 * H]).astype(bfloat16),
        "W1c": np.ascontiguousarray(W_ep1[2 * H : 3 * H]).astype(bfloat16),
        "b_ep1_c": np.asarray(inputs["b_ep1"], np.float32).reshape(H, 1),
        "W_ep2": np.asarray(inputs["W_ep2"], np.float32).astype(bfloat16),
        "b_ep2_c": np.asarray(inputs["b_ep2"], np.float32).reshape(H // 2, 1),
        "W_heads": np.ascontiguousarray(
            np.concatenate(
                [
                    np.asarray(inputs["W_from"], np.float32),
                    np.asarray(inputs["W_to"], np.float32),
                    np.asarray(inputs["W_turn"], np.float32),
                ],
                axis=1,
            )
        ).astype(bfloat16),
        "b_heads_c": np.array(
            [inputs["b_from"][0], inputs["b_to"][0], inputs["b_turn"][0]], np.float32
        ).reshape(3, 1),
        "id_f32": np.eye(P, dtype=np.float32),
        "id_bf16": np.eye(P, dtype=bfloat16),
        "iota": np.tile(np.arange(P, dtype=np.float32), (P, 1)).astype(bfloat16),
        "iotaw": np.tile(np.arange(P, dtype=np.float32).reshape(P, 1), (1, 4 * P)),
        "ones_sq": np.ones((P, P), np.float32).astype(bfloat16),
    }
    for i in range(3):
        shared[f"Wc{i}"] = np.ascontiguousarray(
            np.asarray(inputs["W_conv"], np.float32)[i]
        ).astype(bfloat16)
        shared[f"bc{i}"] = np.broadcast_to(
            np.asarray(inputs["b_conv"], np.float32)[i], (P, H)
        ).copy()
        shared[f"lg{i}"] = np.broadcast_to(
            np.asarray(inputs["ln_g"], np.float32)[i], (P, H)
        ).copy()
        shared[f"lb{i}"] = np.broadcast_to(
            np.asarray(inputs["ln_b"], np.float32)[i], (P, H)
        ).copy()
    for m in in_maps:
        m.update(shared)

    dims = dict(N=N, E=E, EA=EA, nslice=nslice, nblk=nblk, NG=NG, CH=CH,
                Tc=Tc, TT=TT, Ec=Ec)
    return dims, in_maps, meta, perm


def _build(dims):
    f32 = mybir.dt.float32
    bf16 = mybir.dt.bfloat16
    i16 = mybir.dt.int16
    AX = mybir.AxisListType
    OP = mybir.AluOpType
    AF = mybir.ActivationFunctionType

    nslice, nblk, NG, CH, Tc, TT, Ec, EA = (
        dims["nslice"], dims["nblk"], dims["NG"], dims["CH"],
        dims["Tc"], dims["TT"], dims["Ec"], dims["EA"],
    )
    C0 = np.cumsum([0] + list(Tc))
    rg = [list(range(NCORES))]

    nc = bacc.Bacc(
        "TRN2", target_bir_lowering=False, debug=False,
        enable_asserts=False, num_devices=NCORES, num_swdge_queues=NSWQ,
    )

    d_idx16 = nc.dram_tensor("idx16", [nblk * 128, TT * 8], i16, kind="ExternalInput").ap()
    d_cidx16 = (
        None if UB_OHT else
        nc.dram_tensor("cidx16", [nblk * 128, TT * 8], i16, kind="ExternalInput").ap()
    )
    d_colrel = nc.dram_tensor("colrel", [nblk * P, TT], bf16, kind="ExternalInput").ap()
    d_colrelT = nc.dram_tensor("colrelT", [nblk, TT * P], bf16, kind="ExternalInput").ap()
    d_eaT = nc.dram_tensor("eaT", [EA, Ec], bf16, kind="ExternalInput").ap()
    d_xT = nc.dram_tensor("xT", [2, nslice], f32, kind="ExternalInput").ap()
    d_deg = nc.dram_tensor("deg", [P, nblk], f32, kind="ExternalInput").ap()

    din = {}
    consts = [
        ("W_node", [2, H], f32), ("b_node_bc", [P, H], f32),
        ("W_edge", [EA, H], bf16), ("b_edge_c", [H, 1], f32),
        ("W1a", [H, H], bf16), ("W1b", [H, H], bf16), ("W1c", [H, H], bf16),
        ("b_ep1_c", [H, 1], f32), ("W_ep2", [H, H // 2], bf16),
        ("b_ep2_c", [H // 2, 1], f32), ("W_heads", [H // 2, 3], bf16),
        ("b_heads_c", [3, 1], f32), ("id_f32", [P, P], f32),
        ("id_bf16", [P, P], bf16), ("iota", [P, P], bf16),
        ("iotaw", [P, 4 * P], f32), ("ones_sq", [P, P], bf16),
    ]
    for i in range(3):
        consts += [(f"Wc{i}", [H, H], bf16), (f"bc{i}", [P, H], f32),
                   (f"lg{i}", [P, H], f32), (f"lb{i}", [P, H], f32)]
    for name, shp, dt in consts:
        din[name] = nc.dram_tensor(name, shp, dt, kind="ExternalInput").ap()

    d_out = nc.dram_tensor("out", [3, Ec], f32, kind="ExternalOutput").ap()

    # bf16 tables, 128 cols (256B rows): g in [:, 0:64] ([:, 64:128] unused),
    # u as [u_a | u_b].
    ag_g_in = nc.dram_tensor("ag_g_in", [nslice, 2 * H], bf16, kind="Internal").ap()
    g_full = nc.dram_tensor("g_full", [NG, 2 * H], bf16, kind="Internal", addr_space="Shared").ap()
    ag_u_in = nc.dram_tensor("ag_u_in", [nslice, 2 * H], bf16, kind="Internal").ap()
    u_full = nc.dram_tensor("u_full", [NG, 2 * H], bf16, kind="Internal", addr_space="Shared").ap()

    with tile.TileContext(nc) as tc:
        nc.gpsimd.load_library(library_config.mlp)
        cp = tc.alloc_tile_pool(name="consts", bufs=1)
        sb = {}
        for name, shp, dt in consts:
            t = cp.tile(shp, dt, name=f"sb_{name}")
            nc.sync.dma_start(out=t[:], in_=din[name])
            sb[name] = t

        xT_sb = cp.tile([2, nslice], f32, name="xT_sb")
        nc.sync.dma_start(out=xT_sb[:], in_=d_xT)
        deg_sb = cp.tile([P, nblk], f32, name="deg_sb")
        nc.sync.dma_start(out=deg_sb[:], in_=d_deg)
        dinv_sb = cp.tile([P, nblk], f32, name="dinv_sb")
        nc.vector.reciprocal(out=dinv_sb[:], in_=deg_sb[:])
        nc.scalar.sqrt(out=dinv_sb[:], in_=dinv_sb[:])

        hbuf = [cp.tile([P, nblk * H], f32, name=f"hbuf{j}") for j in range(2)]
        gbuf = cp.tile([P, nblk * H], f32, name="gbuf")
        ubuf = cp.tile([P, nblk * H], bf16, name="ubuf")  # resident u_b rows

        # ---- h0 = relu(x @ W_node + b_node)
        with tc.tile_pool(name="p0", bufs=4, space="PSUM") as pp0:
            for b in range(nblk):
                ps = pp0.tile([P, H], f32, name="h0ps")
                nc.tensor.matmul(
                    out=ps[:], lhsT=xT_sb[:, b * P : (b + 1) * P],
                    rhs=sb["W_node"][:], start=True, stop=True,
                )
                hb = hbuf[0][:, b * H : (b + 1) * H]
                nc.vector.tensor_tensor(out=hb, in0=ps[:], in1=sb["b_node_bc"][:], op=OP.add)
                nc.vector.tensor_scalar_max(hb, hb, 0.0)

        # ---- 3 GCN layers
        for li in range(3):
            hc = hbuf[li % 2]
            hn = hbuf[(li + 1) % 2]
            with (
                tc.tile_pool(name=f"gprep{li}", bufs=3, space="PSUM") as gp,
                tc.tile_pool(name=f"gprep_sb{li}", bufs=3) as gs,
            ):
                for b in range(nblk):
                    hT_ps = gp.tile([H, P], f32, name="hT_ps")
                    nc.tensor.matmul(
                        out=hT_ps[:], lhsT=hc[:, b * H : (b + 1) * H],
                        rhs=sb["id_f32"][:], start=True, stop=True,
                    )
                    hT = gs.tile([H, P], bf16, name="hT")
                    nc.any.tensor_copy(out=hT[:], in_=hT_ps[:])
                    g_ps = gp.tile([P, H], f32, name="g_ps")
                    nc.tensor.matmul(
                        out=g_ps[:], lhsT=hT[:], rhs=sb[f"Wc{li}"][:],
                        start=True, stop=True,
                    )
                    gb = gbuf[:, b * H : (b + 1) * H]
                    nc.vector.tensor_scalar(
                        out=gb, in0=g_ps[:], scalar1=dinv_sb[:, b : b + 1],
                        scalar2=None, op0=OP.mult,
                    )
                    g_bf = gs.tile([P, H], bf16, name="g_bf")
                    nc.any.tensor_copy(out=g_bf[:], in_=gb)
                    nc.sync.dma_start(
                        out=ag_g_in[b * P : (b + 1) * P, 0:H], in_=g_bf[:]
                    )

            nc.gpsimd.collective_compute(
                "AllGather", OP.bypass, replica_groups=rg,
                ins=[ag_g_in], outs=[g_full],
            )

            with (
                tc.tile_pool(name=f"sweep{li}", bufs=3) as sp,
                tc.tile_pool(name=f"sweep_ps{li}", bufs=4, space="PSUM") as spp,
                tc.tile_pool(name=f"epi{li}", bufs=2) as ep,
            ):
                for b in range(nblk):
                    idxs = sp.tile([128, TT * 8], i16, name="idxs")
                    nc.sync.dma_start(out=idxs[:], in_=d_idx16[b * 128 : (b + 1) * 128, :])
                    crel = sp.tile([P, TT], bf16, name="crel")
                    nc.sync.dma_start(out=crel[:], in_=d_colrel[b * P : (b + 1) * P, :])
                    gt = sp.tile([P, TT, 2 * H], bf16, name="gt")
                    if TRIM and li == 0 and b < 3:
                        nc.vector.memset(gt[:], 0.0)  # -1-trimmed slots must stay finite
                    for c in range(NCHUNK):
                        nc.gpsimd.dma_gather(
                            out_ap=gt[:, C0[c] : C0[c + 1], :],
                            in_ap=g_full[c * CH : (c + 1) * CH, :],
                            idxs_ap=idxs[:, C0[c] * 8 : C0[c + 1] * 8],
                            num_idxs=Tc[c] * P,
                            num_idxs_reg=Tc[c] * P,
                            elem_size=2 * H,
                            single_packet=False,
                            queue_num=c % NSWQ,
                        )
                    oh = sp.tile([P, TT, P], bf16, name="oh")
                    nc.any.tensor_tensor(
                        out=oh[:],
                        in0=sb["iota"][:].rearrange("p (o f) -> p o f", o=1).to_broadcast([P, TT, P]),
                        in1=crel[:].rearrange("p (t o) -> p t o", o=1).to_broadcast([P, TT, P]),
                        op=OP.is_equal,
                    )
                    agg = spp.tile([P, H], f32, name="agg")
                    for t in range(TT):
                        nc.tensor.matmul(
                            out=agg[:], lhsT=oh[:, t, :], rhs=gt[:, t, 0:H],
                            start=(t == 0), stop=(t == TT - 1),
                        )
                    # epilogue: self-loop, dinv, bias, LN, relu, residual
                    pre = ep.tile([P, H], f32, name="pre")
                    nc.vector.tensor_tensor(
                        out=pre[:], in0=agg[:], in1=gbuf[:, b * H : (b + 1) * H], op=OP.add
                    )
                    nc.vector.tensor_scalar(
                        out=pre[:], in0=pre[:], scalar1=dinv_sb[:, b : b + 1],
                        scalar2=None, op0=OP.mult,
                    )
                    nc.vector.tensor_tensor(out=pre[:], in0=pre[:], in1=sb[f"bc{li}"][:], op=OP.add)
                    mu = ep.tile([P, 1], f32, name="mu")
                    nc.vector.tensor_reduce(out=mu[:], in_=pre[:], axis=AX.X, op=OP.add)
                    nc.vector.tensor_scalar_mul(mu[:], mu[:], 1.0 / H)
                    xc = ep.tile([P, H], f32, name="xc")
                    nc.vector.tensor_scalar(
                        out=xc[:], in0=pre[:], scalar1=mu[:, 0:1], scalar2=None, op0=OP.subtract
                    )
                    sq = ep.tile([P, H], f32, name="sq")
                    vs = ep.tile([P, 1], f32, name="vs")
                    nc.scalar.activation(
                        out=sq[:], in_=xc[:], func=AF.Square, accum_out=vs[:]
                    )
                    nc.vector.tensor_scalar(
                        out=vs[:], in0=vs[:], scalar1=1.0 / H, scalar2=EPS,
                        op0=OP.mult, op1=OP.add,
                    )
                    nc.vector.reciprocal(out=vs[:], in_=vs[:])
                    nc.scalar.sqrt(out=vs[:], in_=vs[:])
                    nc.vector.tensor_scalar(
                        out=xc[:], in0=xc[:], scalar1=vs[:, 0:1], scalar2=None, op0=OP.mult
                    )
                    nc.vector.tensor_tensor(out=xc[:], in0=xc[:], in1=sb[f"lg{li}"][:], op=OP.mult)
                    nc.vector.tensor_tensor(out=xc[:], in0=xc[:], in1=sb[f"lb{li}"][:], op=OP.add)
                    nc.vector.tensor_scalar_max(xc[:], xc[:], 0.0)
                    nc.vector.tensor_tensor(
                        out=hn[:, b * H : (b + 1) * H], in0=xc[:],
                        in1=hc[:, b * H : (b + 1) * H], op=OP.add,
                    )

        # ---- u table [u_a | u_b] (bf16) + AllGather; keep u_b resident
        hfin = hbuf[1]
        with (
            tc.tile_pool(name="uprep", bufs=3, space="PSUM") as up,
            tc.tile_pool(name="uprep_sb", bufs=3) as us,
        ):
            for b in range(nblk):
                hT_ps = up.tile([H, P], f32, name="uhT_ps")
                nc.tensor.matmul(
                    out=hT_ps[:], lhsT=hfin[:, b * H : (b + 1) * H],
                    rhs=sb["id_f32"][:], start=True, stop=True,
                )
                hT = us.tile([H, P], bf16, name="uhT")
                nc.any.tensor_copy(out=hT[:], in_=hT_ps[:])
                u_bf = us.tile([P, 2 * H], bf16, name="u_bf")
                for j, wname in enumerate(("W1a", "W1b")):
                    u_ps = up.tile([P, H], f32, name="u_ps")
                    nc.tensor.matmul(
                        out=u_ps[:], lhsT=hT[:], rhs=sb[wname][:], start=True, stop=True
                    )
                    nc.any.tensor_copy(out=u_bf[:, j * H : (j + 1) * H], in_=u_ps[:])
                nc.vector.tensor_copy(
                    out=ubuf[:, b * H : (b + 1) * H], in_=u_bf[:, H : 2 * H]
                )
                nc.sync.dma_start(out=ag_u_in[b * P : (b + 1) * P, :], in_=u_bf[:])
        nc.gpsimd.collective_compute(
            "AllGather", OP.bypass, replica_groups=rg, ins=[ag_u_in], outs=[u_full]
        )

        # ---- edge MLP (feature-major, bf16)
        tile_groups = [(c, min(4, TT - c)) for c in range(0, TT, 4)]
        with (
            tc.tile_pool(name="fsweep", bufs=3) as fp,
            tc.tile_pool(name="fsweep_ps", bufs=2, space="PSUM") as fpp,
            tc.tile_pool(name="fsweep_ps1", bufs=1, space="PSUM") as fpp1,
        ):
            for b in range(nblk):
                idxs = fp.tile([128, TT * 8], i16, name="fidxs")
                nc.sync.dma_start(out=idxs[:], in_=d_idx16[b * 128 : (b + 1) * 128, :])
                crTz = fp.tile([P, TT * P], bf16, name="crTz")
                if UB_OHT and b < 3:
                    nc.vector.memset(crTz[:], 0.0)  # rows 1-127 stay zero forever
                if UB_OHT:
                    nc.sync.dma_start(out=crTz[0:1, :], in_=d_colrelT[b : b + 1, :])
                uar = fp.tile([P, TT, 2 * H], bf16, name="uar")
                if TRIM and b < 3:
                    nc.vector.memset(uar[:], 0.0)
                for c in range(NCHUNK):
                    nc.gpsimd.dma_gather(
                        out_ap=uar[:, C0[c] : C0[c + 1], :],
                        in_ap=u_full[c * CH : (c + 1) * CH, :],
                        idxs_ap=idxs[:, C0[c] * 8 : C0[c + 1] * 8],
                        num_idxs=Tc[c] * P,
                        num_idxs_reg=Tc[c] * P,
                        elem_size=2 * H,
                        single_packet=False,
                        queue_num=c % NSWQ,
                    )
                if not UB_OHT:
                    cidx = fp.tile([128, TT * 8], i16, name="fcidx")
                    nc.sync.dma_start(out=cidx[:], in_=d_cidx16[b * 128 : (b + 1) * 128, :])
                    ubr = fp.tile([P, TT, 2 * H], bf16, name="ubr")
                    if TRIM and b < 3:
                        nc.vector.memset(ubr[:], 0.0)
                    nc.gpsimd.dma_gather(
                        out_ap=ubr[:],
                        in_ap=ag_u_in[:],
                        idxs_ap=cidx[:],
                        num_idxs=TT * P,
                        num_idxs_reg=TT * P,
                        elem_size=2 * H,
                        single_packet=False,
                    )
                ub_blk = ubuf[:, b * H : (b + 1) * H]
                for c0g, csz in tile_groups:
                    Ech = csz * P
                    col0 = b * TT * P + c0g * P
                    if UB_OHT:
                        # transposed one-hot: broadcast colrelT (in row 0 of the
                        # zeroed crTz tile) across partitions via an all-ones
                        # matmul, then compare against the partition index
                        crT_ps = fpp1.tile([P, 4 * P], f32, name="crT_ps")
                        nc.tensor.matmul(
                            out=crT_ps[:, :Ech], lhsT=sb["ones_sq"][:],
                            rhs=crTz[:, c0g * P : c0g * P + Ech],
                            start=True, stop=True,
                        )
                        ohT = fp.tile([P, 4 * P], bf16, name="ohT")
                        nc.any.tensor_tensor(
                            out=ohT[:, :Ech],
                            in0=sb["iotaw"][:, :Ech],
                            in1=crT_ps[:, :Ech],
                            op=OP.is_equal,
                        )
                    ea = fp.tile([EA, 4 * P], bf16, name="ea")
                    nc.sync.dma_start(out=ea[:, :Ech], in_=d_eaT[:, col0 : col0 + Ech])
                    e_ps = fpp.tile([H, 4 * P], f32, name="e_ps")
                    nc.tensor.matmul(
                        out=e_ps[:, :Ech], lhsT=sb["W_edge"][:], rhs=ea[:, :Ech],
                        start=True, stop=True,
                    )
                    eT = fp.tile([H, 4 * P], bf16, name="eT")
                    nc.scalar.activation(
                        out=eT[:, :Ech], in_=e_ps[:, :Ech], func=AF.Relu,
                        bias=sb["b_edge_c"][:, 0:1],
                    )
                    er_ps = fpp.tile([H, 4 * P], f32, name="er_ps")
                    nc.tensor.matmul(
                        out=er_ps[:, :Ech], lhsT=sb["W1c"][:], rhs=eT[:, :Ech],
                        start=True, stop=False,
                    )
                    for tt in range(csz):
                        t = c0g + tt
                        nc.tensor.matmul(
                            out=er_ps[:, tt * P : (tt + 1) * P], lhsT=uar[:, t, 0:H],
                            rhs=sb["id_bf16"][:], start=False, stop=False,
                            skip_group_check=True,
                        )
                        if UB_OHT:
                            nc.tensor.matmul(
                                out=er_ps[:, tt * P : (tt + 1) * P], lhsT=ub_blk,
                                rhs=ohT[:, tt * P : (tt + 1) * P], start=False,
                                stop=(tt == csz - 1), skip_group_check=True,
                            )
                        else:
                            nc.tensor.matmul(
                                out=er_ps[:, tt * P : (tt + 1) * P],
                                lhsT=ubr[:, t, H : 2 * H],
                                rhs=sb["id_bf16"][:], start=False,
                                stop=(tt == csz - 1), skip_group_check=True,
                            )
                    erT = fp.tile([H, 4 * P], bf16, name="erT")
                    nc.scalar.activation(
                        out=erT[:, :Ech], in_=er_ps[:, :Ech], func=AF.Relu,
                        bias=sb["b_ep1_c"][:, 0:1],
                    )
                    er2_ps = fpp.tile([H // 2, 4 * P], f32, name="er2_ps")
                    nc.tensor.matmul(
                        out=er2_ps[:, :Ech], lhsT=sb["W_ep2"][:], rhs=erT[:, :Ech],
                        start=True, stop=True,
                    )
                    er2 = fp.tile([H // 2, 4 * P], bf16, name="er2")
                    nc.scalar.activation(
                        out=er2[:, :Ech], in_=er2_ps[:, :Ech], func=AF.Relu,
                        bias=sb["b_ep2_c"][:, 0:1],
                    )
                    s_ps = fpp1.tile([3, 4 * P], f32, name="s_ps")
                    nc.tensor.matmul(
                        out=s_ps[:, :Ech], lhsT=sb["W_heads"][:], rhs=er2[:, :Ech],
                        start=True, stop=True,
                    )
                    s_sb = fp.tile([3, 4 * P], f32, name="s_sb")
                    nc.vector.tensor_scalar(
                        out=s_sb[:, :Ech], in0=s_ps[:, :Ech],
                        scalar1=sb["b_heads_c"][:, 0:1], scalar2=None, op0=OP.add,
                    )
                    nc.sync.dma_start(out=d_out[:, col0 : col0 + Ech], in_=s_sb[:, :Ech])
        cp.release()
    nc.compile()
    return nc


def kernel(**inputs):
    dims, in_maps, meta, perm = _prep(inputs)
    nc = _build(dims)
    res = bass_utils.run_bass_kernel_spmd(nc, in_maps, core_ids=list(range(NCORES)))
    E = dims["E"]
    s_sorted = np.zeros((3, E), np.float32)
    for k in range(NCORES):
        flat_pos, sort_idx = meta[k]
        if len(sort_idx):
            s_sorted[:, sort_idx] = res.results[k]["out"][:, flat_pos]
    s = np.zeros((3, E), np.float32)
    s[:, perm] = s_sorted
    return s[0], s[1], s[2]
